# revision 10
# baseline (speedup 1.0000x reference)
"""DeepSeekV2 MoE layer on 8 trn2 NeuronCores (expert-parallel).

Strategy (v4):
  - Host: gate softmax + group-limited top-k routing -> per-expert sorted token
    lists and combine weights (control data only; all heavy FLOPs on device).
  - Device (SPMD over 8 cores, 4 experts each):
      Routed compute is split into two token phases so the cross-core combine
      (ReduceScatter) overlaps compute instead of sitting at the end:
        phase A: each expert's first CAPA=512 (sorted) tokens -> covers all
                 token rows < RA=2048 completely; scatter into y_a (rows<RA)
                 plus a spill scatter into y_b for A-tokens >= RA.
        RS_a:    ReduceScatter(add) of y_a[RA, H] overlaps phase B.
        phase B: remaining tokens (cap CAPB); scatter into y_b only.
        RS_b:    ReduceScatter(add) of y_b[RB, H] overlaps the shared-expert
                 phase, which runs last.
      A tiny warmup ReduceScatter at kernel start absorbs the ~200us
      first-collective cost observed in traces.
      Shared experts (full SI) computed for this core's own 512 output rows;
      out = rs + shared.
  - Host: reassemble row blocks -> [B, S, H].
"""
import sys

import numpy as np

sys.path.insert(0, "/opt/trn_rl_repo")

import concourse.bass as bass
import concourse.mybir as mybir
import concourse.tile as tile
from concourse import bacc
from concourse.bass_utils import run_bass_kernel_spmd

F32 = mybir.dt.float32
FP16 = mybir.dt.float16
I16 = mybir.dt.int16
AF = mybir.ActivationFunctionType
OP = mybir.AluOpType

N_GROUP, TOPK_GROUP, TOP_K = 8, 3, 6
NCORES = 8


def _routing(x, gate_w):
    T, E = x.shape[0], gate_w.shape[0]
    logits = (x @ gate_w.T).astype(np.float64)
    e = np.exp(logits - logits.max(-1, keepdims=True))
    scores = e / e.sum(-1, keepdims=True)
    per_group = E // N_GROUP
    group_scores = scores.reshape(T, N_GROUP, per_group).max(-1)
    order = np.argsort(-group_scores, axis=-1, kind="stable")
    group_mask = np.zeros((T, N_GROUP), bool)
    np.put_along_axis(group_mask, order[:, :TOPK_GROUP], True, axis=1)
    tmp = np.where(np.repeat(group_mask, per_group, axis=1), scores, 0.0)
    order_e = np.argsort(-tmp, axis=-1, kind="stable")
    topk_idx = order_e[:, :TOP_K]
    topk_w = np.take_along_axis(tmp, topk_idx, axis=1)
    topk_w = topk_w / (topk_w.sum(-1, keepdims=True) + 1e-20)
    combine = np.zeros((T, E), np.float32)
    np.put_along_axis(combine, topk_idx, topk_w.astype(np.float32), axis=1)
    return combine


def _wrap16(a):
    """[n] int16 -> [128, n//16] index layout for dma_gather/scatter."""
    return np.tile(a.reshape(-1, 16).T, (8, 1))


def build_kernel(T, H, I, EPC, CAPA, CAPB, RA, SP0, SI, act=AF.Silu,
                 compile_=True):
    KT = H // 128          # contraction tiles over H
    MT = I // 128          # I tiles
    CTA = CAPA // 128
    CTB = CAPB // 128
    RB = T - RA
    SIT = SI // 128
    TOUT = T // NCORES     # own output rows (RA/8 + RB/8)
    TS = TOUT // 128
    NSTR = H // 512        # 512-col strips for shared out / final add
    SPT = (CAPA - SP0) // 128

    nc = bacc.Bacc("TRN2")
    x16 = nc.dram_tensor("x16", [T, H], FP16, kind="ExternalInput")
    xTc = nc.dram_tensor("xTc", [128, KT * TOUT], FP16, kind="ExternalInput")
    w13 = nc.dram_tensor("w13", [EPC, MT, 128, KT * 256], FP16, kind="ExternalInput")
    w2b = nc.dram_tensor("w2b", [EPC, 2, 128, MT * 1024], FP16, kind="ExternalInput")
    sw13 = nc.dram_tensor("sw13", [SIT, 128, KT * 256], FP16, kind="ExternalInput")
    sw2b = nc.dram_tensor("sw2b", [NSTR, 128, SIT * 512], FP16, kind="ExternalInput")
    idxga = nc.dram_tensor("idxga", [EPC, 128, CAPA // 16], I16, kind="ExternalInput")
    idxgb = nc.dram_tensor("idxgb", [EPC, 128, CAPB // 16], I16, kind="ExternalInput")
    idxsa = nc.dram_tensor("idxsa", [EPC, 128, CAPA // 16], I16, kind="ExternalInput")
    idxsp = nc.dram_tensor("idxsp", [EPC, 128, (CAPA - SP0) // 16], I16, kind="ExternalInput")
    idxsb = nc.dram_tensor("idxsb", [EPC, 128, CAPB // 16], I16, kind="ExternalInput")
    gata = nc.dram_tensor("gata", [EPC, 128, CTA], F32, kind="ExternalInput")
    gatb = nc.dram_tensor("gatb", [EPC, 128, CTB], F32, kind="ExternalInput")
    out = nc.dram_tensor("out", [TOUT, H], F32, kind="ExternalOutput")

    y_a = nc.dram_tensor("y_a", [RA + 128, H], FP16)
    y_b = nc.dram_tensor("y_b", [RB + 128, H], FP16)
    rs_a = nc.dram_tensor("rs_a", [RA // NCORES, H], FP16)
    rs_b = nc.dram_tensor("rs_b", [RB // NCORES, H], FP16)
    warm_in = nc.dram_tensor("warm_in", [2048, 512], FP16)
    warm_out = nc.dram_tensor("warm_out", [256, 512], FP16)

    grp = [list(range(NCORES))]

    with tile.TileContext(nc) as tc:
        with (
            tc.tile_pool(name="const", bufs=1) as const,
            tc.tile_pool(name="persist", bufs=1) as persist,
            tc.tile_pool(name="xgtp", bufs=2) as xgtp,
            tc.tile_pool(name="gp", bufs=2) as gp,
            tc.tile_pool(name="w13p", bufs=2) as w13p,
            tc.tile_pool(name="w2p", bufs=2) as w2p,
            tc.tile_pool(name="ybp", bufs=2) as ybp,
            tc.tile_pool(name="s13p", bufs=2) as s13p,
            tc.tile_pool(name="s2p", bufs=1) as s2p,
            tc.tile_pool(name="small", bufs=2) as small,
            tc.tile_pool(name="psum", bufs=2, space="PSUM") as psum,
        ):
            # ---------------- constants ------------------------------------
            iga = const.tile([128, EPC, CAPA // 16], I16)
            nc.sync.dma_start(iga[:], idxga.rearrange("e p c -> p e c"))
            igb = const.tile([128, EPC, CAPB // 16], I16)
            nc.sync.dma_start(igb[:], idxgb.rearrange("e p c -> p e c"))
            isa = const.tile([128, EPC, CAPA // 16], I16)
            nc.sync.dma_start(isa[:], idxsa.rearrange("e p c -> p e c"))
            isp = const.tile([128, EPC, (CAPA - SP0) // 16], I16)
            nc.sync.dma_start(isp[:], idxsp.rearrange("e p c -> p e c"))
            isb = const.tile([128, EPC, CAPB // 16], I16)
            nc.sync.dma_start(isb[:], idxsb.rearrange("e p c -> p e c"))
            ga_sb = const.tile([128, EPC, CTA], F32)
            nc.sync.dma_start(ga_sb[:], gata.rearrange("e p c -> p e c"))
            gb_sb = const.tile([128, EPC, CTB], F32)
            nc.sync.dma_start(gb_sb[:], gatb.rearrange("e p c -> p e c"))

            # ---------------- warmup collective + zero fills ---------------
            ztile = const.tile([128, 2048], FP16)
            nc.vector.memset(ztile[:], 0.0)
            for b in range(16):
                nc.scalar.dma_start(warm_in[b * 128:(b + 1) * 128, :],
                                    ztile[:, :512])
            nc.gpsimd.collective_compute(
                "ReduceScatter", OP.add, replica_groups=grp,
                ins=[warm_in[:]], outs=[warm_out[:]])
            for b in range(RA // 128):
                nc.scalar.dma_start(y_a[b * 128:(b + 1) * 128, :], ztile[:])
            for b in range(RB // 128):
                nc.scalar.dma_start(y_b[b * 128:(b + 1) * 128, :], ztile[:])

            # shared-expert input tokens (used at the end)
            xtc_sb = persist.tile([128, KT, TOUT], FP16)
            xtc_view = xTc.rearrange("p (k t) -> p k t", t=TOUT)
            for k in range(KT):
                nc.scalar.dma_start(xtc_sb[:, k:k + 1, :], xtc_view[:, k:k + 1, :])
            gs = persist.tile([128, SIT, TOUT], FP16)

            # ---------------- routed experts: two token phases -------------
            def routed_phase(cap, ct_n, idxg, idxs_main, y_main, gat_sb,
                             idx_spill=None, spill_ct0=0):
                for e in range(EPC):
                    xgt = xgtp.tile([128, KT, cap], FP16, tag="xgt")
                    nc.gpsimd.dma_gather(
                        xgt[:], x16[:], idxg[:, e, :], cap, cap, H,
                        transpose=True)
                    g = gp.tile([128, MT, cap], FP16, tag="g")
                    for m in range(MT):
                        w13t = w13p.tile([128, KT, 256], FP16, tag="w13t")
                        nc.sync.dma_start(
                            w13t[:], w13[e, m].rearrange("p (k c) -> p k c", c=256))
                        p1 = psum.tile([128, 512], F32, tag="p1")
                        p3 = psum.tile([128, 512], F32, tag="p3")
                        for k in range(KT):
                            nc.tensor.matmul(p1[:, :cap], w13t[:, k, :128],
                                             xgt[:, k, :],
                                             start=(k == 0), stop=(k == KT - 1))
                        for k in range(KT):
                            nc.tensor.matmul(p3[:, :cap], w13t[:, k, 128:],
                                             xgt[:, k, :],
                                             start=(k == 0), stop=(k == KT - 1))
                        nc.scalar.activation(g[:, m, :], p1[:, :cap], act)
                        nc.vector.tensor_tensor(g[:, m, :], g[:, m, :],
                                                p3[:, :cap], OP.mult)
                    ybs = [ybp.tile([128, ct_n, 512], FP16, tag=f"yb{s}",
                                    name=f"yb{s}")
                           for s in range(4)]
                    for half in range(2):
                        w2t = w2p.tile([128, MT, 1024], FP16, tag="w2t")
                        nc.sync.dma_start(
                            w2t[:], w2b[e, half].rearrange("p (k c) -> p k c", c=1024))
                        for ct in range(ct_n):
                            p4a = psum.tile([128, 512], F32, tag="p4a")
                            p4b = psum.tile([128, 512], F32, tag="p4b")
                            for k2 in range(MT):
                                nc.tensor.matmul(p4a[:], g[:, k2, ct * 128:(ct + 1) * 128],
                                                 w2t[:, k2, :512],
                                                 start=(k2 == 0), stop=(k2 == MT - 1))
                                nc.tensor.matmul(p4b[:], g[:, k2, ct * 128:(ct + 1) * 128],
                                                 w2t[:, k2, 512:],
                                                 start=(k2 == 0), stop=(k2 == MT - 1))
                            nc.vector.tensor_tensor(
                                ybs[2 * half][:, ct, :], p4a[:],
                                gat_sb[:, e, ct:ct + 1].to_broadcast([128, 512]),
                                OP.mult)
                            nc.vector.tensor_tensor(
                                ybs[2 * half + 1][:, ct, :], p4b[:],
                                gat_sb[:, e, ct:ct + 1].to_broadcast([128, 512]),
                                OP.mult)
                    for s in range(4):
                        nc.gpsimd.dma_scatter_add(
                            y_main[:, s * 512:(s + 1) * 512], ybs[s][:],
                            idxs_main[:, e, :], cap, cap, 512, elem_step=H)
                        if idx_spill is not None:
                            nc.gpsimd.dma_scatter_add(
                                y_b[:, s * 512:(s + 1) * 512],
                                ybs[s][:, spill_ct0:, :], idx_spill[:, e, :],
                                cap - spill_ct0 * 128, cap - spill_ct0 * 128,
                                512, elem_step=H)

            routed_phase(CAPA, CTA, iga, isa, y_a, ga_sb,
                         idx_spill=isp, spill_ct0=SP0 // 128)
            nc.gpsimd.collective_compute(
                "ReduceScatter", OP.add, replica_groups=grp,
                ins=[y_a[0:RA, :]], outs=[rs_a[:]])
            routed_phase(CAPB, CTB, igb, isb, y_b, gb_sb)
            nc.gpsimd.collective_compute(
                "ReduceScatter", OP.add, replica_groups=grp,
                ins=[y_b[0:RB, :]], outs=[rs_b[:]])

            # ---------------- shared experts (own rows) --------------------
            for sm in range(SIT):
                s13 = s13p.tile([128, KT, 256], FP16, tag="s13")
                nc.scalar.dma_start(
                    s13[:], sw13[sm].rearrange("p (k c) -> p k c", c=256))
                p1 = psum.tile([128, 512], F32, tag="p1")
                p3 = psum.tile([128, 512], F32, tag="p3")
                for k in range(KT):
                    nc.tensor.matmul(p1[:, :TOUT], s13[:, k, :128], xtc_sb[:, k, :],
                                     start=(k == 0), stop=(k == KT - 1))
                for k in range(KT):
                    nc.tensor.matmul(p3[:, :TOUT], s13[:, k, 128:], xtc_sb[:, k, :],
                                     start=(k == 0), stop=(k == KT - 1))
                nc.scalar.activation(gs[:, sm, :], p1[:, :TOUT], act)
                nc.vector.tensor_tensor(gs[:, sm, :], gs[:, sm, :], p3[:, :TOUT],
                                        OP.mult)

            # shared out per 512-col strip + combine with rs halves
            half_ts = TS // 2  # first half of own rows come from rs_a
            for s in range(NSTR):
                s2 = s2p.tile([128, SIT, 512], FP16, tag="s2")
                nc.scalar.dma_start(
                    s2[:], sw2b[s].rearrange("p (k c) -> p k c", c=512))
                for ts in range(TS):
                    po = psum.tile([128, 512], F32, tag="p4a")
                    for k2 in range(SIT):
                        nc.tensor.matmul(po[:], gs[:, k2, ts * 128:(ts + 1) * 128],
                                         s2[:, k2, :],
                                         start=(k2 == 0), stop=(k2 == SIT - 1))
                    rst = small.tile([128, 512], FP16, tag="rst")
                    if ts < half_ts:
                        src = rs_a[ts * 128:(ts + 1) * 128, s * 512:(s + 1) * 512]
                    else:
                        src = rs_b[(ts - half_ts) * 128:(ts - half_ts + 1) * 128,
                                   s * 512:(s + 1) * 512]
                    nc.scalar.dma_start(rst[:], src)
                    ott = small.tile([128, 512], F32, tag="ott")
                    nc.vector.tensor_tensor(ott[:], po[:], rst[:], OP.add)
                    nc.sync.dma_start(
                        out[ts * 128:(ts + 1) * 128, s * 512:(s + 1) * 512], ott[:])

    if compile_:
        nc.compile()
    else:
        nc.insert_library_loads()
    return nc


def host_prep(hidden_states, gate_weight, w1, w2, w3, sw1, sw2, sw3):
    B, S, H = hidden_states.shape
    T = B * S
    E, I = w1.shape[0], w1.shape[1]
    SI = sw1.shape[0]
    EPC = E // NCORES
    KT, MT, SIT = H // 128, I // 128, SI // 128
    NSTR = H // 512

    x = np.ascontiguousarray(hidden_states.reshape(T, H), dtype=np.float32)
    combine = _routing(x, gate_weight.astype(np.float32))
    tok_lists = [np.nonzero(combine[:, e])[0] for e in range(E)]
    counts = np.array([len(t) for t in tok_lists])

    CAPA = 512
    RA = T // 2
    counts_a = np.array([(t < RA).sum() for t in tok_lists])
    # phase A must contain every token < RA and at least CAPA tokens total
    while RA > 1024 and (counts_a.max() > CAPA or counts.min() < CAPA):
        RA -= 1024
        counts_a = np.array([(t < RA).sum() for t in tok_lists])
    assert counts_a.max() <= CAPA and counts.min() >= CAPA, (
        counts_a.max(), counts.min())
    CAPB = max(128, ((counts.max() - CAPA + 127) // 128) * 128)
    SP0 = int(counts_a.min()) // 128 * 128
    RB = T - RA

    x16 = x.astype(np.float16)
    xT = x.T  # [H, T] view

    s1 = sw1.T.reshape(KT, 128, SIT, 128).transpose(2, 1, 0, 3)
    s3 = sw3.T.reshape(KT, 128, SIT, 128).transpose(2, 1, 0, 3)
    sw13 = np.ascontiguousarray(
        np.concatenate([s1, s3], axis=-1).reshape(SIT, 128, -1), dtype=np.float16)
    sw2b = np.ascontiguousarray(
        sw2.T.reshape(SIT, 128, NSTR, 512).transpose(2, 1, 0, 3).reshape(NSTR, 128, -1),
        dtype=np.float16)

    in_maps = []
    for c in range(NCORES):
        els = list(range(c * EPC, (c + 1) * EPC))
        idxga = np.zeros((EPC, 128, CAPA // 16), np.int16)
        idxgb = np.zeros((EPC, 128, CAPB // 16), np.int16)
        idxsa = np.zeros((EPC, 128, CAPA // 16), np.int16)
        idxsp = np.zeros((EPC, 128, (CAPA - SP0) // 16), np.int16)
        idxsb = np.zeros((EPC, 128, CAPB // 16), np.int16)
        gata = np.zeros((EPC, 128, CAPA // 128), np.float32)
        gatb = np.zeros((EPC, 128, CAPB // 128), np.float32)
        for j, e in enumerate(els):
            toks = tok_lists[e]
            na = int(counts_a[e])
            ta = toks[:CAPA]                      # full, >= CAPA guaranteed
            tb = toks[CAPA:]
            nb = len(tb)
            # gather lists
            idxga[j] = _wrap16(ta.astype(np.int16))
            bpad = np.zeros(CAPB, np.int16)
            bpad[:nb] = tb
            idxgb[j] = _wrap16(bpad)
            # scatter A -> y_a rows (<RA); entries >=RA dumped to pad row RA
            sa = np.full(CAPA, RA, np.int16)
            sa[:na] = ta[:na]
            idxsa[j] = _wrap16(sa)
            # scatter A spill -> y_b rows; positions [SP0, CAPA)
            sp = np.full(CAPA - SP0, RB, np.int16)  # pad row of y_b
            sel = ta[SP0:]
            sp[sel >= RA] = (sel[sel >= RA] - RA).astype(np.int16)
            idxsp[j] = _wrap16(sp)
            # scatter B -> y_b rows; pad entries dumped to pad row RB
            sb = np.full(CAPB, RB, np.int16)
            sb[:nb] = (tb - RA).astype(np.int16)
            idxsb[j] = _wrap16(sb)
            # gate values
            gva = combine[ta, e].astype(np.float32)
            gata[j] = gva.reshape(-1, 128).T
            gvb = np.zeros(CAPB, np.float32)
            gvb[:nb] = combine[tb, e]
            gatb[j] = gvb.reshape(-1, 128).T
        w13c = np.empty((EPC, MT, 128, KT * 256), np.float16)
        w2c = np.empty((EPC, 2, 128, MT * 1024), np.float16)
        for j, e in enumerate(els):
            a1 = w1[e].T.reshape(KT, 128, MT, 128).transpose(2, 1, 0, 3)
            a3 = w3[e].T.reshape(KT, 128, MT, 128).transpose(2, 1, 0, 3)
            w13c[j] = np.concatenate([a1, a3], axis=-1).reshape(MT, 128, -1)
            w2c[j] = (w2[e].T.reshape(MT, 128, 2, 1024)
                      .transpose(2, 1, 0, 3).reshape(2, 128, -1))
        own_rows = np.concatenate([
            np.arange(c * RA // NCORES, (c + 1) * RA // NCORES),
            np.arange(RA + c * RB // NCORES, RA + (c + 1) * RB // NCORES)])
        xTc = np.ascontiguousarray(
            xT[:, own_rows].reshape(KT, 128, len(own_rows))
            .transpose(1, 0, 2).reshape(128, -1), dtype=np.float16)
        in_maps.append({
            "x16": x16, "xTc": xTc,
            "w13": w13c, "w2b": w2c,
            "sw13": sw13, "sw2b": sw2b,
            "idxga": idxga, "idxgb": idxgb,
            "idxsa": idxsa, "idxsp": idxsp, "idxsb": idxsb,
            "gata": gata, "gatb": gatb,
        })
    cfg = dict(T=T, H=H, I=I, EPC=EPC, CAPA=CAPA, CAPB=CAPB, RA=RA, SP0=SP0,
               SI=SI)
    return in_maps, cfg


def kernel(**inputs):
    inputs = {k: np.asarray(v) for k, v in inputs.items()}
    hs = inputs["hidden_states"]
    B, S, H = hs.shape
    in_maps, cfg = host_prep(
        hs, inputs["gate_weight"], inputs["w1"], inputs["w2"], inputs["w3"],
        inputs["sw1"], inputs["sw2"], inputs["sw3"])
    nc = build_kernel(**cfg)
    res = run_bass_kernel_spmd(nc, in_maps, list(range(NCORES)))
    T = B * S
    RA = cfg["RA"]
    RB = T - RA
    y = np.empty((T, H), np.float32)
    for c in range(NCORES):
        o = res.results[c]["out"]
        y[c * RA // NCORES:(c + 1) * RA // NCORES] = o[:RA // NCORES]
        y[RA + c * RB // NCORES:RA + (c + 1) * RB // NCORES] = o[RA // NCORES:]
    return y.reshape(B, S, H).astype(np.float32)


if __name__ == "__main__":
    pass
